# revision 7
# baseline (speedup 1.0000x reference)
"""Trainium2 Bass kernel for nn_DetectionHead (VoteNet-style detection head).

Self-contained: builds an 8-core SPMD Bass/Tile kernel, shards the M=128
clusters across cores (interleaved mod 8), replicates FPS + NMS, and
AllGathers the per-core box logits for the final NMS pass.

v2: fine-grained cluster groups (2 clusters each) so the per-cluster MLP
overlaps the sequential FPS from iteration ~15 instead of 63; single-matmul
d2 masks via a host-side [x;y;z;|p|^2] tensor; one AllGather for the first
14 clusters hidden under tail compute plus a tiny final AllGather.

kernel(**inputs) takes the full unsharded inputs and returns the full
[128, 6] output.
"""

import numpy as np

NCORES = 8
N = 4096          # points
C = 128           # feature channels
M = 128           # clusters
MC = M // NCORES  # clusters per core (16)
NJ = 32           # FPS free-dim (N = 128 * NJ)
RADIUS = 0.5
THR = RADIUS * RADIUS   # 0.25 (d2 < THR)
NMS_THR = 0.25
BIG = 1.0e7
NMS_ITERS = 6
CHUNK = 512
NCHUNK = N // CHUNK       # 8

_cache = {}


def _build(debug=False):
    import concourse.bacc as bacc
    import concourse.tile as tile
    import concourse.mybir as mybir
    import concourse.bass_isa as bass_isa

    F32 = mybir.dt.float32
    F32R = mybir.dt.float32r
    BF16 = mybir.dt.bfloat16
    I32 = mybir.dt.int32
    ALU = mybir.AluOpType
    ACTF = mybir.ActivationFunctionType
    AX = mybir.AxisListType

    nc = bacc.Bacc("TRN2", target_bir_lowering=False, debug=False,
                   num_devices=NCORES)

    # ---- DRAM I/O ----
    d_pts96 = nc.dram_tensor("pts96", [128, 96], F32, kind="ExternalInput")
    d_pT = nc.dram_tensor("pT", [3, N], F32, kind="ExternalInput")
    d_pT4 = nc.dram_tensor("pT4", [4, N], F32, kind="ExternalInput")
    d_featT = nc.dram_tensor("featT", [C, N], F32, kind="ExternalInput")
    d_W1a = nc.dram_tensor("W1a", [3, C], F32, kind="ExternalInput")
    d_W1am2 = nc.dram_tensor("W1am2", [3, C], F32, kind="ExternalInput")
    d_W1b = nc.dram_tensor("W1b", [C, C], F32, kind="ExternalInput")
    d_W2 = nc.dram_tensor("W2", [C, C], F32, kind="ExternalInput")
    d_W3 = nc.dram_tensor("W3", [C, C], F32, kind="ExternalInput")
    d_W4 = nc.dram_tensor("W4", [C, C], F32, kind="ExternalInput")
    d_Wf = nc.dram_tensor("Wf", [C, 7], F32, kind="ExternalInput")
    d_b1r = nc.dram_tensor("b1r", [1, C], F32, kind="ExternalInput")
    d_b2c = nc.dram_tensor("b2c", [C, 1], F32, kind="ExternalInput")
    d_b3c = nc.dram_tensor("b3c", [C, 1], F32, kind="ExternalInput")
    d_b4c = nc.dram_tensor("b4c", [C, 1], F32, kind="ExternalInput")
    d_bfr = nc.dram_tensor("bfr", [1, 7], F32, kind="ExternalInput")
    d_sel16 = nc.dram_tensor("sel16", [128, MC], F32, kind="ExternalInput")

    d_out = nc.dram_tensor("out", [M, 6], F32, kind="ExternalOutput")

    from contextlib import ExitStack
    es = ExitStack()
    with tile.TileContext(nc) as tc:
        cp = es.enter_context(tc.tile_pool(name="const", bufs=1))
        # ---- constant / persistent tiles ----
        pts96 = cp.tile([128, 96], F32)
        pT = cp.tile([3, N], F32)
        pT4 = cp.tile([4, N], F32)
        featT = cp.tile([C, N], F32)
        P3 = cp.tile([C, N], F32R)
        W1a = cp.tile([3, C], F32)
        W1am2 = cp.tile([3, C], F32)
        W1b = cp.tile([C, C], F32)
        W2r = cp.tile([C, C], BF16)
        W3r = cp.tile([C, C], F32R)
        W4r = cp.tile([C, C], F32R)
        Wfr = cp.tile([C, 7], F32R)
        W2 = cp.tile([C, C], F32)
        W3 = cp.tile([C, C], F32)
        W4 = cp.tile([C, C], F32)
        Wf = cp.tile([C, 7], F32)
        b1r = cp.tile([1, C], F32)
        b2c = cp.tile([C, 1], F32)
        b3c = cp.tile([C, 1], F32)
        b4c = cp.tile([C, 1], F32)
        bfr = cp.tile([1, 7], F32)
        sel16 = cp.tile([128, MC], F32)
        ident = cp.tile([128, 128], F32)
        ident_i = cp.tile([128, 128], I32)
        ones_1x128 = cp.tile([1, 128], F32)
        ones_1x16 = cp.tile([1, MC], F32)
        NB8 = cp.tile([8, 8 * 128], BF16)
        NB8_i = cp.tile([8, 8 * 128], I32)
        ER = cp.tile([8, 8 * 128], F32)
        ER_i = cp.tile([8, 8 * 128], I32)
        centers_all = cp.tile([128, 3], F32)
        # FPS tiles: SoA point data + per-iteration state
        Q4 = cp.tile([128, 128], F32)      # [x|y|z|pp] blocks of NJ
        pm2 = cp.tile([128, 96], F32)      # p - c
        sqt = cp.tile([128, 96], F32)
        minA = cp.tile([128, NJ], F32)
        minB = cp.tile([128, NJ], F32)
        ft1 = cp.tile([128, NJ], F32)
        mq = cp.tile([128, 128], F32)
        rowmax = cp.tile([128, 1], F32)
        gb = cp.tile([128, 1], F32)
        cand = cp.tile([128, 4], F32)
        candw = cp.tile([128, 4], F32)
        selqA = cp.tile([128, 4], F32)
        selqB = cp.tile([128, 4], F32)
        iota_neg = cp.tile([128, NJ], F32)
        iota_i = cp.tile([128, NJ], I32)
        G = cp.tile([C, MC], F32)
        BTmine = cp.tile([7, MC], F32)
        # NMS tiles
        S14 = cp.tile([14, 128], F32)
        BX = cp.tile([128, 14], F32)
        PR = cp.tile([128, 8], F32)
        TPs = cp.tile([8, 128], F32)
        P_s = cp.tile([128, 128], BF16)
        keep = cp.tile([128, 1], BF16)
        keepf = cp.tile([128, 1], F32)
        lo3 = cp.tile([128, 3], F32)
        hi3 = cp.tile([128, 3], F32)
        vol = cp.tile([128, 1], F32)
        outt = cp.tile([128, 6], F32)

        # ---- input DMA ----
        nc.sync.dma_start(pts96[:], d_pts96.ap())
        nc.sync.dma_start(pT[:], d_pT.ap())
        nc.sync.dma_start(pT4[:], d_pT4.ap())
        nc.sync.dma_start(featT[:], d_featT.ap())
        nc.sync.dma_start(W1a[:], d_W1a.ap())
        nc.sync.dma_start(W1am2[:], d_W1am2.ap())
        nc.sync.dma_start(W1b[:], d_W1b.ap())
        nc.sync.dma_start(W2[:], d_W2.ap())
        nc.sync.dma_start(W3[:], d_W3.ap())
        nc.sync.dma_start(W4[:], d_W4.ap())
        nc.sync.dma_start(Wf[:], d_Wf.ap())
        nc.sync.dma_start(b1r[:], d_b1r.ap())
        nc.sync.dma_start(b2c[:], d_b2c.ap())
        nc.sync.dma_start(b3c[:], d_b3c.ap())
        nc.sync.dma_start(b4c[:], d_b4c.ap())
        nc.sync.dma_start(bfr[:], d_bfr.ap())
        nc.sync.dma_start(sel16[:], d_sel16.ap())

        # ---- constants ----
        nc.gpsimd.iota(ident_i[:], pattern=[[1, 128]], base=0,
                       channel_multiplier=-1)
        nc.vector.tensor_scalar(ident[:], ident_i[:], 0, None,
                                op0=ALU.is_equal)
        nc.vector.memset(ones_1x128[:], 1.0)
        nc.vector.memset(ones_1x16[:], 1.0)
        nc.gpsimd.iota(NB8_i[:].rearrange("p (j c) -> p j c", c=128),
                       pattern=[[1, 8], [0, 128]], base=0,
                       channel_multiplier=-1)
        nc.vector.tensor_scalar(NB8[:], NB8_i[:], 0, -BIG,
                                op0=ALU.is_equal, op1=ALU.mult)
        nc.gpsimd.iota(ER_i[:].rearrange("p (j c) -> p j c", c=128),
                       pattern=[[1, 8], [0, 128]], base=0,
                       channel_multiplier=-1)
        nc.vector.tensor_scalar(ER[:], ER_i[:], 0, None, op0=ALU.is_equal)
        nc.vector.tensor_copy(W2r[:], W2[:])
        nc.vector.tensor_copy(W3r[:], W3[:])
        nc.vector.tensor_copy(W4r[:], W4[:])
        nc.vector.tensor_copy(Wfr[:], Wf[:])

        # ---- FPS static data: Q4 = [x|y|z|pp] in SoA blocks of NJ ----
        nc.vector.tensor_copy(
            Q4[:, 0:96].rearrange("p (c j) -> p c j", j=NJ),
            pts96[:].rearrange("p (j c) -> p c j", c=3))
        nc.vector.tensor_mul(sqt[:], Q4[:, 0:96], Q4[:, 0:96])
        nc.vector.tensor_reduce(
            Q4[:, 96:128],
            sqt[:].rearrange("p (c j) -> p j c", j=NJ),
            axis=AX.X, op=ALU.add)
        nc.vector.memset(minA[:], BIG)
        nc.vector.memset(centers_all[:], 0.0)
        nc.gpsimd.iota(iota_i[:], pattern=[[1, NJ]], base=0,
                       channel_multiplier=NJ)
        nc.vector.tensor_scalar(iota_neg[:], iota_i[:], -1.0, None,
                                op0=ALU.mult)

        # ================= FPS =================
        def selq_of(t):
            return selqA if t % 2 == 0 else selqB

        def record_center(t):
            # center t: copy the broadcast selq row to partition t
            nc.sync.dma_start(centers_all[t:t + 1, 0:3],
                              selq_of(t)[0:1, 0:3])

        def fps_select(t, md):
            """Select center t as the argmax of md into selq slot t%2.
            rowmax must already hold the row maxes of md."""
            nc.gpsimd.partition_all_reduce(gb[:], rowmax[:], channels=128,
                                           reduce_op=bass_isa.ReduceOp.max)
            # per-partition candidate (independent of gb; hides under hop1)
            nc.vector.scalar_tensor_tensor(
                out=mq[:, 0:96].rearrange("p (c j) -> p c j", j=NJ),
                in0=md[:].unsqueeze(1).broadcast_to([128, 3, NJ]),
                scalar=rowmax[:],
                in1=Q4[:, 0:96].rearrange("p (c j) -> p c j", j=NJ),
                op0=ALU.is_ge, op1=ALU.mult)
            nc.vector.tensor_reduce(
                cand[:, 0:3], mq[:, 0:96].rearrange("p (c j) -> p c j", j=NJ),
                axis=AX.X, op=ALU.add)
            # keep only the globally-winning partition's candidate
            nc.vector.scalar_tensor_tensor(
                candw[:, 0:3].unsqueeze(2),
                rowmax[:].unsqueeze(1).broadcast_to([128, 3, 1]), gb[:],
                cand[:, 0:3].unsqueeze(2),
                op0=ALU.is_ge, op1=ALU.mult)
            nc.gpsimd.partition_all_reduce(
                selq_of(t)[:, 0:3], candw[:, 0:3], channels=128,
                reduce_op=bass_isa.ReduceOp.add)
            record_center(t)

        # t = 0: argmax of -n selects point 0 through the same machinery
        nc.vector.tensor_reduce(rowmax[:], iota_neg[:], axis=AX.X, op=ALU.max)
        fps_select(0, iota_neg)

        def fps_iter(t):
            """Update min_d with center t-1, then select center t."""
            sq = selq_of(t - 1)
            mo, mn = (minA, minB) if t % 2 == 1 else (minB, minA)
            # d_new = |p - c|^2 ; min_d = min(min_d, d_new)
            nc.vector.tensor_tensor(
                pm2[:].rearrange("p (j c) -> p j c", c=3),
                pts96[:].rearrange("p (j c) -> p j c", c=3),
                sq[:, 0:3].unsqueeze(1).broadcast_to([128, NJ, 3]),
                op=ALU.subtract)
            nc.vector.tensor_mul(sqt[:], pm2[:], pm2[:])
            nc.vector.tensor_reduce(
                ft1[:], sqt[:].rearrange("p (j c) -> p j c", c=3),
                axis=AX.X, op=ALU.add)
            nc.vector.tensor_tensor(mn[:], ft1[:], mo[:], op=ALU.min)
            nc.vector.tensor_reduce(rowmax[:], mn[:], axis=AX.X,
                                    op=ALU.max)
            fps_select(t, mn)

        # ---- pools for the overlapped per-cluster pipeline ----
        # PSUM budget (8 banks): p3(2, closed early) -> sc2 + mlp(2x2) + bx1
        p3_es = ExitStack()
        p3_psum = p3_es.enter_context(
            tc.tile_pool(name="p3_psum", bufs=2, space="PSUM"))
        sc_es = ExitStack()
        grp_pool = sc_es.enter_context(tc.tile_pool(name="grp", bufs=3))
        mlp_es = ExitStack()
        h1_pool = mlp_es.enter_context(tc.tile_pool(name="h1", bufs=8))
        gp_pool = mlp_es.enter_context(tc.tile_pool(name="gp", bufs=3))
        bx_es = ExitStack()
        bx_pool = bx_es.enter_context(tc.tile_pool(name="bx", bufs=2))
        dram = es.enter_context(tc.tile_pool(name="dram", bufs=1,
                                             space="DRAM"))
        # PSUM pools for the steady state open only after p3_psum closes
        # (banks are reserved at pool-open time): see the fps loop below.
        psum_pools = {}

        def open_steady_psum():
            psum_pools["sc"] = sc_es.enter_context(
                tc.tile_pool(name="sc_psum", bufs=2, space="PSUM"))
            psum_pools["mlp"] = mlp_es.enter_context(
                tc.tile_pool(name="mlp_psum", bufs=2, space="PSUM"))
            psum_pools["bx"] = bx_es.enter_context(
                tc.tile_pool(name="bx_psum", bufs=1, space="PSUM"))

        # P3 = W1b^T @ featT + (-2 W1a)^T @ pT  (cluster independent),
        # issued as pump items so it overlaps the first FPS iterations.
        def p3_chunk(ci):
            sl = slice(ci * CHUNK, (ci + 1) * CHUNK)
            ps = p3_psum.tile([C, CHUNK], F32, tag="p3ps")
            nc.tensor.matmul(ps[:], W1b[:], featT[:, sl], start=True,
                             stop=False)
            nc.tensor.matmul(ps[:], W1am2[:], pT[:, sl], start=False,
                             stop=True)
            nc.scalar.copy(P3[:, sl], ps[:])

        # per-group state (rotating pool tiles)
        gstate = {}

        def stage_a1(g, j0, nj):
            st = gstate[g] = {}
            st["cmine"] = grp_pool.tile([8, 3], F32, tag="cmine",
                                        name=f"cmine{g}")
            st["negthr"] = grp_pool.tile([8, 1], F32, tag="negthr",
                                         name=f"negthr{g}")
            st["ctm"] = grp_pool.tile([3, 8], F32, tag="ctm",
                                      name=f"ctm{g}")
            st["ctm2a"] = grp_pool.tile([4, 8], F32, tag="ctm2a",
                                        name=f"ctm2a{g}")
            st["U2b"] = grp_pool.tile([C, 8], F32, tag="U2b",
                                      name=f"U2b{g}")
            st["mask"] = grp_pool.tile([8, N], BF16, tag="mask",
                                       name=f"mask{g}")
            ps_cm = psum_pools["sc"].tile([8, 3], F32, tag="sc")
            nc.tensor.matmul(ps_cm[0:nj, :], sel16[:, j0:j0 + nj],
                             centers_all[:], start=True, stop=True)
            nc.scalar.copy(st["cmine"][0:nj, :], ps_cm[0:nj, :])

        def stage_a2(g, j0, nj):
            st = gstate[g]
            cmine, negthr = st["cmine"], st["negthr"]
            ctm, ctm2a, U2b = st["ctm"], st["ctm2a"], st["U2b"]
            # |c|^2 - THR per cluster (bias for the mask evict)
            tmp3 = grp_pool.tile([8, 3], F32, tag="tmp3")
            nc.vector.tensor_mul(tmp3[0:nj, :], cmine[0:nj, :],
                                 cmine[0:nj, :])
            nc.vector.tensor_reduce(negthr[0:nj, :], tmp3[0:nj, :],
                                    axis=AX.X, op=ALU.add)
            nc.vector.tensor_scalar(negthr[0:nj, :], negthr[0:nj, :],
                                    -THR, None, op0=ALU.add)
            ps_ctm = psum_pools["sc"].tile([3, 8], F32, tag="sc")
            nc.tensor.transpose(ps_ctm[:, 0:nj], cmine[0:nj, :],
                                ident[0:nj, 0:nj])
            nc.scalar.copy(ctm[:, 0:nj], ps_ctm[:, 0:nj])
            nc.vector.memset(ctm2a[0:4, 0:nj], 1.0)
            nc.scalar.mul(ctm2a[0:3, 0:nj], ps_ctm[:, 0:nj], -2.0)
            ps_u = psum_pools["sc"].tile([C, 8], F32, tag="sc")
            nc.tensor.matmul(ps_u[:, 0:nj], W1a[:], ctm[:, 0:nj],
                             start=True, stop=False)
            nc.tensor.matmul(ps_u[:, 0:nj], b1r[:], ones_1x16[:, 0:nj],
                             start=False, stop=True)
            nc.scalar.copy(U2b[:, 0:nj], ps_u[:, 0:nj])

        def stage_m(g, nj, ci):
            """Mask chunk: relu(|p|^2 - 2 c.p + |c|^2 - THR) in bf16."""
            st = gstate[g]
            sl = slice(ci * CHUNK, (ci + 1) * CHUNK)
            ps_m = psum_pools["sc"].tile([8, CHUNK], F32, tag="sc")
            nc.tensor.matmul(ps_m[0:nj, :], st["ctm2a"][:, 0:nj],
                             pT4[:, sl], start=True, stop=True)
            nc.scalar.activation(st["mask"][0:nj, sl], ps_m[0:nj, :],
                                 ACTF.Relu, bias=st["negthr"][0:nj, :],
                                 scale=1.0)

        def mlp_pair_a(g, nj, jl, pair):
            """TENSOR/ACT half of one (cluster, 1024-pt) tile."""
            st = gstate[g]
            ps2 = psum_pools["mlp"].tile([C, 2 * CHUNK], F32, tag="ps2")
            for half in range(2):
                ci = 2 * pair + half
                sl = slice(ci * CHUNK, (ci + 1) * CHUNK)
                qsl = slice(half * CHUNK, (half + 1) * CHUNK)
                h1 = h1_pool.tile([C, CHUNK], BF16, tag="h1")
                nc.scalar.activation(h1[:], P3[:, sl], ACTF.Relu,
                                     bias=st["U2b"][:, jl:jl + 1],
                                     scale=1.0)
                nc.tensor.matmul(ps2[:, qsl], W2r[:], h1[:],
                                 start=True, stop=False)
                nc.tensor.matmul(ps2[:, qsl],
                                 NB8[0:nj, jl * 128:(jl + 1) * 128],
                                 st["mask"][0:nj, sl], start=False,
                                 stop=True)
            rr = h1_pool.tile([C, 2 * CHUNK], BF16, tag="rr",
                              name=f"rr{g}_{jl}_{pair}")
            nc.scalar.copy(rr[:], ps2[:])
            st[("rr", jl, pair)] = rr

        def mlp_pair_b(g, nj, jl, pair):
            """Deferred DVE max-reduce of the pair's bf16 tile."""
            st = gstate[g]
            if pair == 0:
                st[("gp", jl)] = gp_pool.tile([C, 4], F32, tag="gparts",
                                              name=f"gp{g}_{jl}")
            rr = st.pop(("rr", jl, pair))
            nc.vector.tensor_reduce(st[("gp", jl)][:, pair:pair + 1], rr[:],
                                    axis=AX.X, op=ALU.max)

        def uf(g, j0, jl):
            st = gstate[g]
            nc.vector.tensor_reduce(
                G[:, j0 + jl:j0 + jl + 1], st[("gp", jl)][:],
                axis=AX.X, op=ALU.max)

        def boxes_group(j0, nj):
            """G cols -> box logits for clusters [j0, j0+nj)."""
            grel = bx_pool.tile([C, 8], F32R, tag="grel")
            nc.scalar.activation(grel[:, 0:nj], G[:, j0:j0 + nj],
                                 ACTF.Relu, bias=b2c[:], scale=1.0)
            ps_g3 = psum_pools["bx"].tile([C, 8], F32, tag="bx")
            nc.tensor.matmul(ps_g3[:, 0:nj], W3r[:], grel[:, 0:nj],
                             start=True, stop=True)
            g3 = bx_pool.tile([C, 8], F32R, tag="g3s")
            nc.scalar.activation(g3[:, 0:nj], ps_g3[:, 0:nj], ACTF.Relu,
                                 bias=b3c[:], scale=1.0)
            ps_g4 = psum_pools["bx"].tile([C, 8], F32, tag="bx")
            nc.tensor.matmul(ps_g4[:, 0:nj], W4r[:], g3[:, 0:nj],
                             start=True, stop=True)
            g4 = bx_pool.tile([C, 8], F32R, tag="g4s")
            nc.scalar.activation(g4[:, 0:nj], ps_g4[:, 0:nj], ACTF.Relu,
                                 bias=b4c[:], scale=1.0)
            ps_bx = psum_pools["bx"].tile([7, 8], F32, tag="bx")
            nc.tensor.matmul(ps_bx[:, 0:nj], Wfr[:], g4[:, 0:nj],
                             start=True, stop=False)
            nc.tensor.matmul(ps_bx[:, 0:nj], bfr[:], ones_1x16[:, 0:nj],
                             start=False, stop=True)
            nc.scalar.copy(BTmine[:, j0:j0 + nj], ps_bx[:, 0:nj])

        from collections import deque
        pending = deque()

        # startup work: P3 chunks
        for ci in range(NCHUNK):
            pending.append(lambda ci=ci: p3_chunk(ci))

        def enqueue_group(g, j0, j1, with_boxes=True):
            nj = j1 - j0
            pending.append(lambda: stage_a1(g, j0, nj))
            pending.append(lambda: stage_a2(g, j0, nj))
            for ci in range(NCHUNK):
                pending.append(lambda ci=ci: stage_m(g, nj, ci))
            # interleave pair A (TE/ACT) and deferred B (DVE) items
            seq = []
            for jl in range(nj):
                for pair in range(4):
                    seq.append(("A", jl, pair))
            for idx, (_, jl, pair) in enumerate(seq):
                pending.append(
                    lambda jl=jl, pair=pair: mlp_pair_a(g, nj, jl, pair))
                # deferred reduce from ~2 items back
                if idx >= 2:
                    pjl, ppair = seq[idx - 2][1], seq[idx - 2][2]
                    pending.append(
                        lambda jl=pjl, pair=ppair: mlp_pair_b(g, nj, jl,
                                                              pair))
                    if ppair == 3:
                        pending.append(lambda jl=pjl: uf(g, j0, jl))
            for idx in (len(seq) - 2, len(seq) - 1):
                pjl, ppair = seq[idx][1], seq[idx][2]
                pending.append(
                    lambda jl=pjl, pair=ppair: mlp_pair_b(g, nj, jl, pair))
                if ppair == 3:
                    pending.append(lambda jl=pjl: uf(g, j0, jl))
            if with_boxes:
                pending.append(lambda: boxes_group(j0, nj))

        def pump(n):
            for _ in range(n):
                if pending:
                    pending.popleft()()

        # groups of 2 clusters: group g covers j in [2g, 2g+2), ready at
        # t = 16g+15.  The last two are singletons (j=14 @ t=119, j=15 @
        # t=127) with their boxes deferred to one even-width pass at the end.
        GROUP_AT = {}
        for g in range(7):
            GROUP_AT[16 * g + 15] = (g, 2 * g, 2 * g + 2, True)
        GROUP_AT[119] = (7, 14, 15, False)

        p3_closed = False
        for t in range(1, M):
            fps_iter(t)
            if t in GROUP_AT:
                g, j0, j1, wb = GROUP_AT[t]
                enqueue_group(g, j0, j1, with_boxes=wb)
            pump(2)
            if not p3_closed and t >= 8:
                p3_es.close()
                open_steady_psum()
                p3_closed = True
        if not p3_closed:
            p3_es.close()
            open_steady_psum()

        # ---- tail: AllGather 1 (j=0..13) hidden under remaining work ----
        bnc_in1 = dram.tile([7, 14], F32, name="bnc_in1")
        bnc_out1 = dram.tile([NCORES, 7 * 14], F32, name="bnc_out1")
        nc.sync.dma_start(bnc_in1[:], BTmine[:, 0:14])
        nc.gpsimd.collective_compute(
            "AllGather", mybir.AluOpType.bypass,
            replica_groups=[list(range(NCORES))],
            ins=[bnc_in1[:].opt()],
            outs=[bnc_out1[:].opt()],
        )

        # last cluster (j=15) + deferred boxes for j=14..16
        enqueue_group(8, 15, 16, with_boxes=False)
        pending.append(lambda: boxes_group(14, 2))
        while pending:
            pump(1)

        bnc_in2 = dram.tile([7, 2], F32, name="bnc_in2")
        bnc_out2 = dram.tile([NCORES, 7 * 2], F32, name="bnc_out2")
        nc.sync.dma_start(bnc_in2[:], BTmine[:, 14:16])
        nc.gpsimd.collective_compute(
            "AllGather", mybir.AluOpType.bypass,
            replica_groups=[list(range(NCORES))],
            ins=[bnc_in2[:].opt()],
            outs=[bnc_out2[:].opt()],
        )

        bx_es.close()
        mlp_es.close()
        sc_es.close()

        # reassemble: global cluster m = 8*j + k lives at
        # bounce_out[k, c*nj + (j - j0)]
        BTall = cp.tile([7, 128], F32)
        nc.sync.dma_start(
            BTall[:, 0:112].rearrange("c (j k) -> c j k", k=NCORES),
            bnc_out1[:].rearrange("k (c j) -> c j k", j=14),
        )
        nc.sync.dma_start(
            BTall[:, 112:128].rearrange("c (j k) -> c j k", k=NCORES),
            bnc_out2[:].rearrange("k (c j) -> c j k", j=2),
        )

        # ================= NMS =================
        nms_es = ExitStack()
        nms_psum = nms_es.enter_context(
            tc.tile_pool(name="nms_psum", bufs=1, space="PSUM"))
        # S7 = sigmoid(logits); BX = [sig | logits] transposed
        nc.scalar.activation(S14[0:7, :], BTall[:], ACTF.Sigmoid)
        ps_bxall = nms_psum.tile([128, 14], F32, tag="bxall")
        nc.tensor.transpose(ps_bxall[:, 0:7], S14[0:7, :], ident[0:7, 0:7])
        nc.tensor.transpose(ps_bxall[:, 7:14], BTall[:], ident[0:7, 0:7])
        nc.vector.tensor_copy(BX[:], ps_bxall[:])
        # cols of BX: 0 score-sig, 1..3 center, 4..6 dims, 7 score-logit
        nc.vector.scalar_tensor_tensor(lo3[:], BX[:, 4:7], -0.5, BX[:, 1:4],
                                       op0=ALU.mult, op1=ALU.add)
        nc.vector.scalar_tensor_tensor(hi3[:], BX[:, 4:7], 0.5, BX[:, 1:4],
                                       op0=ALU.mult, op1=ALU.add)
        nc.vector.tensor_mul(vol[:], BX[:, 4:5], BX[:, 5:6])
        nc.vector.tensor_mul(vol[:], vol[:], BX[:, 6:7])
        # PR = [lo3 | hi3 | vol | score-logit]
        nc.vector.tensor_copy(PR[:, 0:3], lo3[:])
        nc.vector.tensor_copy(PR[:, 3:6], hi3[:])
        nc.vector.tensor_copy(PR[:, 6:7], vol[:])
        nc.vector.tensor_copy(PR[:, 7:8], BX[:, 7:8])
        ps_tp = nms_psum.tile([8, 128], F32, tag="tp")
        nc.tensor.transpose(ps_tp[:], PR[:], ident[:])
        nc.vector.tensor_copy(TPs[:], ps_tp[:])
        # broadcast all 8 rows: psumB[:, r*128:(r+1)*128] = row r over parts
        psB = nms_psum.tile([128, 8 * 128], F32, tag="psB")
        for r in range(8):
            nc.tensor.matmul(psB[:, r * 128:(r + 1) * 128],
                             ER[:, r * 128:(r + 1) * 128],
                             TPs[:], start=True, stop=True)

        def colB(r):
            return psB[:, r * 128:(r + 1) * 128]

        wrk = nms_es.enter_context(tc.tile_pool(name="nms_wrk", bufs=1))
        inter = wrk.tile([128, 128], F32, tag="inter")
        tmpA = wrk.tile([128, 128], F32, tag="tmpA")
        tmpB = wrk.tile([128, 128], F32, tag="tmpB")
        for c in range(3):
            # min(hi_i, hi_j)
            nc.vector.tensor_scalar(tmpA[:], colB(3 + c), hi3[:, c:c + 1],
                                    None, op0=ALU.min)
            # max(lo_i, lo_j)
            nc.vector.tensor_scalar(tmpB[:], colB(c), lo3[:, c:c + 1], None,
                                    op0=ALU.max)
            # w = relu(minhi - maxlo)
            nc.vector.scalar_tensor_tensor(tmpA[:], tmpB[:], -1.0, tmpA[:],
                                           op0=ALU.mult, op1=ALU.add)
            nc.vector.tensor_scalar_max(tmpA[:], tmpA[:], 0.0)
            if c == 0:
                nc.vector.tensor_copy(inter[:], tmpA[:])
            else:
                nc.vector.tensor_mul(inter[:], inter[:], tmpA[:])
        # volsum = vol_i + vol_j + 1e-8
        nc.vector.tensor_scalar(tmpB[:], colB(6), vol[:], 1e-8, op0=ALU.add,
                                op1=ALU.add)
        # D = volsum - inter
        nc.vector.scalar_tensor_tensor(tmpB[:], inter[:], -1.0, tmpB[:],
                                       op0=ALU.mult, op1=ALU.add)
        # P_iou = (4*inter > D)
        nc.vector.scalar_tensor_tensor(tmpA[:], inter[:], 1.0 / NMS_THR,
                                       tmpB[:], op0=ALU.mult, op1=ALU.is_gt)
        # P_score[i,j] = score_j < score_i
        nc.vector.tensor_scalar(tmpB[:], colB(7), BX[:, 7:8], None,
                                op0=ALU.is_lt)
        nc.vector.tensor_mul(P_s[:], tmpA[:], tmpB[:])
        # Jacobi fixpoint: keep_j = !any_i P[i,j] keep_i  (bf16 matmuls)
        nc.vector.memset(keep[:], 1.0)
        ps_k = nms_psum.tile([128, 1], F32, tag="kps")
        for it in range(NMS_ITERS):
            nc.tensor.matmul(ps_k[:], P_s[:], keep[:], start=True, stop=True)
            nc.vector.tensor_scalar(keep[:], ps_k[:], 0.5, None,
                                    op0=ALU.is_lt)
        nc.vector.tensor_copy(keepf[:], keep[:])
        # out = coords * keep
        nc.vector.tensor_scalar(outt[:], BX[:, 1:7], keepf[:], None,
                                op0=ALU.mult)
        nc.sync.dma_start(d_out.ap(), outt[:])

        nms_es.close()
        es.close()

    nc.compile()
    return nc


def _prep_inputs(vote_points, vote_features, W1, b1, W2, b2, W3, b3, W4, b4,
                 Wf, bf):
    """Pure layout transforms of the full inputs -> per-core input maps."""
    f32 = np.float32
    pts = np.ascontiguousarray(vote_points, dtype=f32)
    feat = np.ascontiguousarray(vote_features, dtype=f32)
    pT = pts.T.copy()
    pT4 = np.vstack([pT, (pts * pts).sum(axis=1)[None, :]]).astype(f32)
    base = {
        "pts96": pts.reshape(128, 96).copy(),
        "pT": pT,
        "pT4": pT4,
        "featT": feat.T.copy(),
        "W1a": np.ascontiguousarray(W1[:3], f32),
        "W1am2": np.ascontiguousarray(W1[:3] * -2.0, f32),
        "W1b": np.ascontiguousarray(W1[3:], f32),
        "W2": np.ascontiguousarray(W2, f32),
        "W3": np.ascontiguousarray(W3, f32),
        "W4": np.ascontiguousarray(W4, f32),
        "Wf": np.ascontiguousarray(Wf, f32),
        "b1r": np.ascontiguousarray(b1, f32).reshape(1, C),
        "b2c": np.ascontiguousarray(b2, f32).reshape(C, 1),
        "b3c": np.ascontiguousarray(b3, f32).reshape(C, 1),
        "b4c": np.ascontiguousarray(b4, f32).reshape(C, 1),
        "bfr": np.ascontiguousarray(bf, f32).reshape(1, 7),
    }
    in_maps = []
    for k in range(NCORES):
        m = dict(base)
        sel = np.zeros((128, MC), f32)
        for j in range(MC):
            sel[NCORES * j + k, j] = 1.0
        m["sel16"] = sel
        in_maps.append(m)
    return in_maps


def kernel(**inputs):
    from concourse.bass_utils import run_bass_kernel_spmd

    if "nc" not in _cache:
        _cache["nc"] = _build(debug=False)
    nc = _cache["nc"]
    in_maps = _prep_inputs(**inputs)
    res = run_bass_kernel_spmd(nc, in_maps, core_ids=list(range(NCORES)))
    out = np.asarray(res.results[0]["out"], dtype=np.float32)
    return out


# revision 13
# speedup vs baseline: 1.2626x; 1.2626x over previous
"""Trainium2 Bass kernel for nn_DetectionHead (VoteNet-style detection head).

Self-contained: builds an 8-core SPMD Bass/Tile kernel, shards the M=128
clusters across cores (interleaved mod 8), replicates FPS + NMS, and
AllGathers the per-core box logits for the final NMS pass.

v2: fine-grained cluster groups (2 clusters each) so the per-cluster MLP
overlaps the sequential FPS from iteration ~15 instead of 63; single-matmul
d2 masks via a host-side [x;y;z;|p|^2] tensor; one AllGather for the first
14 clusters hidden under tail compute plus a tiny final AllGather.

kernel(**inputs) takes the full unsharded inputs and returns the full
[128, 6] output.
"""

import numpy as np

NCORES = 8
N = 4096          # points
C = 128           # feature channels
M = 128           # clusters
MC = M // NCORES  # clusters per core (16)
NJ = 32           # FPS free-dim (N = 128 * NJ)
RADIUS = 0.5
THR = RADIUS * RADIUS   # 0.25 (d2 < THR)
NMS_THR = 0.25
BIG = 1.0e7
NMS_ITERS = 6
CHUNK = 512
NCHUNK = N // CHUNK       # 8

_cache = {}


def _build(debug=False):
    import concourse.bacc as bacc
    import concourse.tile as tile
    import concourse.mybir as mybir
    import concourse.bass_isa as bass_isa

    F32 = mybir.dt.float32
    F32R = mybir.dt.float32r
    BF16 = mybir.dt.bfloat16
    I32 = mybir.dt.int32
    ALU = mybir.AluOpType
    ACTF = mybir.ActivationFunctionType
    AX = mybir.AxisListType

    nc = bacc.Bacc("TRN2", target_bir_lowering=False, debug=False,
                   num_devices=NCORES)

    # ---- DRAM I/O ----
    d_pts96 = nc.dram_tensor("pts96", [128, 96], F32, kind="ExternalInput")
    d_pT = nc.dram_tensor("pT", [3, N], F32, kind="ExternalInput")
    d_featT = nc.dram_tensor("featT", [C, N], F32, kind="ExternalInput")
    d_W1a = nc.dram_tensor("W1a", [3, C], F32, kind="ExternalInput")
    d_W1am2 = nc.dram_tensor("W1am2", [3, C], F32, kind="ExternalInput")
    d_W1b = nc.dram_tensor("W1b", [C, C], F32, kind="ExternalInput")
    d_W2 = nc.dram_tensor("W2", [C, C], F32, kind="ExternalInput")
    d_W3 = nc.dram_tensor("W3", [C, C], F32, kind="ExternalInput")
    d_W4 = nc.dram_tensor("W4", [C, C], F32, kind="ExternalInput")
    d_Wf = nc.dram_tensor("Wf", [C, 7], F32, kind="ExternalInput")
    d_b1r = nc.dram_tensor("b1r", [1, C], F32, kind="ExternalInput")
    d_b2c = nc.dram_tensor("b2c", [C, 1], F32, kind="ExternalInput")
    d_b3c = nc.dram_tensor("b3c", [C, 1], F32, kind="ExternalInput")
    d_b4c = nc.dram_tensor("b4c", [C, 1], F32, kind="ExternalInput")
    d_bfr = nc.dram_tensor("bfr", [1, 7], F32, kind="ExternalInput")
    d_sel16 = nc.dram_tensor("sel16", [128, MC], F32, kind="ExternalInput")
    d_wsel = nc.dram_tensor("wsel", [128, 129], F32, kind="ExternalInput")

    d_out = nc.dram_tensor("out", [M, 6], F32, kind="ExternalOutput")

    from contextlib import ExitStack
    es = ExitStack()
    with tile.TileContext(nc) as tc:
        cp = es.enter_context(tc.tile_pool(name="const", bufs=1))
        # ---- constant / persistent tiles ----
        pts96 = cp.tile([128, 96], F32)
        pT = cp.tile([3, N], F32)
        pTr = cp.tile([3, N], F32R)
        featT = cp.tile([C, N], F32)
        featTr = cp.tile([C, N], F32R)
        P3 = cp.tile([C, N], F32R)
        W1a = cp.tile([3, C], F32)
        W1am2 = cp.tile([3, C], F32)
        W1b = cp.tile([C, C], F32)
        W1br = cp.tile([C, C], F32R)
        W1am2r = cp.tile([3, C], F32R)
        W2r = cp.tile([C, C], BF16)
        W3r = cp.tile([C, C], F32R)
        W4r = cp.tile([C, C], F32R)
        Wfr = cp.tile([C, 7], F32R)
        W2 = cp.tile([C, C], F32)
        W3 = cp.tile([C, C], F32)
        W4 = cp.tile([C, C], F32)
        Wf = cp.tile([C, 7], F32)
        b1r = cp.tile([1, C], F32)
        b2c = cp.tile([C, 1], F32)
        b3c = cp.tile([C, 1], F32)
        b4c = cp.tile([C, 1], F32)
        bfr = cp.tile([1, 7], F32)
        sel16 = cp.tile([128, MC], F32)
        wsel = cp.tile([128, 129], F32)
        ringd = [cp.tile([128, NJ], F32, name=f"ringd{j}") for j in range(MC)]
        ident = cp.tile([128, 128], F32)
        ident_i = cp.tile([128, 128], I32)
        ones_1x128 = cp.tile([1, 128], F32)
        ones_1x16 = cp.tile([1, MC], F32)
        NB8 = cp.tile([8, 8 * 128], BF16)
        NB8_i = cp.tile([8, 8 * 128], I32)
        ER = cp.tile([8, 8 * 128], F32)
        ER_i = cp.tile([8, 8 * 128], I32)
        centers_all = cp.tile([128, 3], F32)
        negthr_c = cp.tile([128, 1], F32)
        # FPS tiles: SoA point data + per-iteration state
        Q4 = cp.tile([128, 128], F32)      # [x|y|z|pp] blocks of NJ
        pm2 = cp.tile([128, 96], F32)      # p - c
        sqt = cp.tile([128, 96], F32)
        minA = cp.tile([128, NJ], F32)
        minB = cp.tile([128, NJ], F32)
        ft1 = cp.tile([128, NJ], F32)
        mq = cp.tile([128, 128], F32)
        rowmax = cp.tile([128, 1], F32)
        gb = cp.tile([128, 1], F32)
        cand = cp.tile([128, 4], F32)
        candw = cp.tile([128, 4], F32)
        selqA = cp.tile([128, 4], F32)
        selqB = cp.tile([128, 4], F32)
        iota_neg = cp.tile([128, NJ], F32)
        iota_i = cp.tile([128, NJ], I32)
        G = cp.tile([C, MC], F32)
        BTmine = cp.tile([7, MC], F32)
        # NMS tiles
        S14 = cp.tile([14, 128], F32)
        BX = cp.tile([128, 14], F32)
        PR = cp.tile([128, 8], F32)
        TPs = cp.tile([8, 128], F32)
        P_s = cp.tile([128, 128], BF16)
        keep = cp.tile([128, 1], BF16)
        keepf = cp.tile([128, 1], F32)
        lo3 = cp.tile([128, 3], F32)
        hi3 = cp.tile([128, 3], F32)
        vol = cp.tile([128, 1], F32)
        outt = cp.tile([128, 6], F32)

        # ---- input DMA ----
        nc.sync.dma_start(pts96[:], d_pts96.ap())
        nc.sync.dma_start(pT[:], d_pT.ap())
        nc.sync.dma_start(featT[:], d_featT.ap())
        nc.sync.dma_start(W1a[:], d_W1a.ap())
        nc.sync.dma_start(W1am2[:], d_W1am2.ap())
        nc.sync.dma_start(W1b[:], d_W1b.ap())
        nc.sync.dma_start(W2[:], d_W2.ap())
        nc.sync.dma_start(W3[:], d_W3.ap())
        nc.sync.dma_start(W4[:], d_W4.ap())
        nc.sync.dma_start(Wf[:], d_Wf.ap())
        nc.sync.dma_start(b1r[:], d_b1r.ap())
        nc.sync.dma_start(b2c[:], d_b2c.ap())
        nc.sync.dma_start(b3c[:], d_b3c.ap())
        nc.sync.dma_start(b4c[:], d_b4c.ap())
        nc.sync.dma_start(bfr[:], d_bfr.ap())
        nc.sync.dma_start(sel16[:], d_sel16.ap())
        nc.sync.dma_start(wsel[:], d_wsel.ap())

        # ---- constants ----
        nc.gpsimd.iota(ident_i[:], pattern=[[1, 128]], base=0,
                       channel_multiplier=-1)
        nc.vector.tensor_scalar(ident[:], ident_i[:], 0, None,
                                op0=ALU.is_equal)
        nc.vector.memset(ones_1x128[:], 1.0)
        nc.vector.memset(ones_1x16[:], 1.0)
        nc.gpsimd.iota(NB8_i[:].rearrange("p (j c) -> p j c", c=128),
                       pattern=[[1, 8], [0, 128]], base=0,
                       channel_multiplier=-1)
        nc.vector.tensor_scalar(NB8[:], NB8_i[:], 0, -BIG,
                                op0=ALU.is_equal, op1=ALU.mult)
        nc.gpsimd.iota(ER_i[:].rearrange("p (j c) -> p j c", c=128),
                       pattern=[[1, 8], [0, 128]], base=0,
                       channel_multiplier=-1)
        nc.vector.tensor_scalar(ER[:], ER_i[:], 0, None, op0=ALU.is_equal)
        nc.vector.tensor_copy(W1br[:], W1b[:])
        nc.vector.tensor_copy(W1am2r[:], W1am2[:])
        nc.vector.tensor_copy(W2r[:], W2[:])
        nc.vector.tensor_copy(W3r[:], W3[:])
        nc.vector.tensor_copy(W4r[:], W4[:])
        nc.vector.tensor_copy(Wfr[:], Wf[:])

        # ---- FPS static data: Q4 = [x|y|z|pp] in SoA blocks of NJ ----
        nc.vector.tensor_copy(
            Q4[:, 0:96].rearrange("p (c j) -> p c j", j=NJ),
            pts96[:].rearrange("p (j c) -> p c j", c=3))
        nc.vector.tensor_mul(sqt[:], Q4[:, 0:96], Q4[:, 0:96])
        nc.vector.tensor_reduce(
            Q4[:, 96:128],
            sqt[:].rearrange("p (c j) -> p j c", j=NJ),
            axis=AX.X, op=ALU.add)
        nc.vector.memset(minA[:], BIG)
        nc.vector.memset(negthr_c[:], -THR)
        for j in range(MC):
            nc.vector.memset(ringd[j][:], 0.0)
        nc.vector.memset(centers_all[:], 0.0)
        nc.gpsimd.iota(iota_i[:], pattern=[[1, NJ]], base=0,
                       channel_multiplier=NJ)
        nc.vector.tensor_scalar(iota_neg[:], iota_i[:], -1.0, None,
                                op0=ALU.mult)

        # ================= FPS =================
        def selq_of(t):
            return selqA if t % 2 == 0 else selqB

        def record_center(t):
            # center t: copy the broadcast selq row to partition t
            nc.sync.dma_start(centers_all[t:t + 1, 0:3],
                              selq_of(t)[0:1, 0:3])

        def fps_select(t, md):
            """Select center t as the argmax of md into selq slot t%2.
            rowmax must already hold the row maxes of md."""
            nc.gpsimd.partition_all_reduce(gb[:], rowmax[:], channels=128,
                                           reduce_op=bass_isa.ReduceOp.max)
            # per-partition candidate (independent of gb; hides under hop1)
            nc.vector.scalar_tensor_tensor(
                out=mq[:, 0:96].rearrange("p (c j) -> p c j", j=NJ),
                in0=md[:].unsqueeze(1).broadcast_to([128, 3, NJ]),
                scalar=rowmax[:],
                in1=Q4[:, 0:96].rearrange("p (c j) -> p c j", j=NJ),
                op0=ALU.is_ge, op1=ALU.mult)
            nc.vector.tensor_reduce(
                cand[:, 0:3], mq[:, 0:96].rearrange("p (c j) -> p c j", j=NJ),
                axis=AX.X, op=ALU.add)
            # keep only the globally-winning partition's candidate
            nc.vector.scalar_tensor_tensor(
                candw[:, 0:3].unsqueeze(2),
                rowmax[:].unsqueeze(1).broadcast_to([128, 3, 1]), gb[:],
                cand[:, 0:3].unsqueeze(2),
                op0=ALU.is_ge, op1=ALU.mult)
            nc.gpsimd.partition_all_reduce(
                selq_of(t)[:, 0:3], candw[:, 0:3], channels=128,
                reduce_op=bass_isa.ReduceOp.add)
            record_center(t)

        # t = 0: argmax of -n selects point 0 through the same machinery
        nc.vector.tensor_reduce(rowmax[:], iota_neg[:], axis=AX.X, op=ALU.max)
        fps_select(0, iota_neg)

        def fps_iter(t):
            """Update min_d with center t-1, then select center t."""
            sq = selq_of(t - 1)
            mo, mn = (minA, minB) if t % 2 == 1 else (minB, minA)
            # d_new = |p - c|^2 ; min_d = min(min_d, d_new)
            nc.vector.tensor_tensor(
                pm2[:].rearrange("p (j c) -> p j c", c=3),
                pts96[:].rearrange("p (j c) -> p j c", c=3),
                sq[:, 0:3].unsqueeze(1).broadcast_to([128, NJ, 3]),
                op=ALU.subtract)
            nc.vector.tensor_mul(sqt[:], pm2[:], pm2[:])
            nc.vector.tensor_reduce(
                ft1[:], sqt[:].rearrange("p (j c) -> p j c", c=3),
                axis=AX.X, op=ALU.add)
            nc.vector.tensor_tensor(mn[:], ft1[:], mo[:], op=ALU.min)
            nc.vector.tensor_reduce(rowmax[:], mn[:], axis=AX.X,
                                    op=ALU.max)
            fps_select(t, mn)
            # capture |p - c_{t-1}|^2 into cluster (t-1)//8's ring slot when
            # this core owns global center t-1 (wsel is the per-core gate)
            nc.vector.scalar_tensor_tensor(
                ringd[(t - 1) // 8][:], ft1[:], wsel[:, t:t + 1],
                ringd[(t - 1) // 8][:], op0=ALU.mult, op1=ALU.add)

        # ---- pools for the overlapped per-cluster pipeline ----
        # PSUM budget (8 banks): p3(2, closed early) -> sc2 + mlp(2x2) + bx1
        p3_es = ExitStack()
        p3_psum = p3_es.enter_context(
            tc.tile_pool(name="p3_psum", bufs=2, space="PSUM"))
        sc_es = ExitStack()
        grp_pool = sc_es.enter_context(tc.tile_pool(name="grp", bufs=3))
        mlp_es = ExitStack()
        h1_pool = mlp_es.enter_context(tc.tile_pool(name="h1", bufs=4))
        rr_pool = mlp_es.enter_context(tc.tile_pool(name="rr", bufs=4))
        mrow_pool = mlp_es.enter_context(tc.tile_pool(name="mrow", bufs=4))
        mrows = {}
        gp_pool = mlp_es.enter_context(tc.tile_pool(name="gp", bufs=3))
        bx_es = ExitStack()
        bx_pool = bx_es.enter_context(tc.tile_pool(name="bx", bufs=2))
        dram = es.enter_context(tc.tile_pool(name="dram", bufs=1,
                                             space="DRAM"))
        # PSUM pools for the steady state open only after p3_psum closes
        # (banks are reserved at pool-open time): see the fps loop below.
        psum_pools = {}

        def open_steady_psum():
            psum_pools["sc"] = sc_es.enter_context(
                tc.tile_pool(name="sc_psum", bufs=2, space="PSUM"))
            psum_pools["mlp"] = mlp_es.enter_context(
                tc.tile_pool(name="mlp_psum", bufs=2, space="PSUM"))
            psum_pools["bx"] = bx_es.enter_context(
                tc.tile_pool(name="bx_psum", bufs=1, space="PSUM"))

        # P3 = W1b^T @ featT + (-2 W1a)^T @ pT  (cluster independent),
        # issued as pump items so it overlaps the first FPS iterations.
        def p3_chunk(ci):
            sl = slice(ci * CHUNK, (ci + 1) * CHUNK)
            ps = p3_psum.tile([C, CHUNK], F32, tag="p3ps")
            nc.scalar.copy(featTr[:, sl], featT[:, sl])
            nc.scalar.copy(pTr[:, sl], pT[:, sl])
            nc.tensor.matmul(ps[:], W1br[:], featTr[:, sl], start=True,
                             stop=False)
            nc.tensor.matmul(ps[:], W1am2r[:], pTr[:, sl], start=False,
                             stop=True)
            nc.scalar.copy(P3[:, sl], ps[:])

        # per-group state (rotating pool tiles)
        gstate = {}

        def stage_a1(g, j0, nj):
            st = gstate[g] = {}
            st["cmine"] = grp_pool.tile([8, 3], F32, tag="cmine",
                                        name=f"cmine{g}")
            st["ctm"] = grp_pool.tile([3, 8], F32, tag="ctm",
                                      name=f"ctm{g}")
            st["U2b"] = grp_pool.tile([C, 8], F32, tag="U2b",
                                      name=f"U2b{g}")
            ps_cm = psum_pools["sc"].tile([8, 3], F32, tag="sc")
            nc.tensor.matmul(ps_cm[0:nj, :], sel16[:, j0:j0 + nj],
                             centers_all[:], start=True, stop=True)
            nc.scalar.copy(st["cmine"][0:nj, :], ps_cm[0:nj, :])

        def stage_a2(g, j0, nj):
            st = gstate[g]
            cmine = st["cmine"]
            ctm, U2b = st["ctm"], st["U2b"]
            ps_ctm = psum_pools["sc"].tile([3, 8], F32, tag="sc")
            nc.tensor.transpose(ps_ctm[:, 0:nj], cmine[0:nj, :],
                                ident[0:nj, 0:nj])
            nc.scalar.copy(ctm[:, 0:nj], ps_ctm[:, 0:nj])
            ps_u = psum_pools["sc"].tile([C, 8], F32, tag="sc")
            nc.tensor.matmul(ps_u[:, 0:nj], W1a[:], ctm[:, 0:nj],
                             start=True, stop=False)
            nc.tensor.matmul(ps_u[:, 0:nj], b1r[:], ones_1x16[:, 0:nj],
                             start=False, stop=True)
            nc.scalar.copy(U2b[:, 0:nj], ps_u[:, 0:nj])

        def mask_relu(j):
            """relu(d2 - THR) in bf16, partition layout [128, NJ]."""
            mr = grp_pool.tile([128, NJ], BF16, tag="mrelu",
                               name=f"mrelu{j}")
            gstate[("mr", j)] = mr
            nc.scalar.activation(mr[:], ringd[j][:], ACTF.Relu,
                                 bias=negthr_c[:], scale=1.0)

        def mask_flat(j):
            """Flatten [128, NJ] -> [1, N] via a DRAM round-trip."""
            mr = gstate.pop(("mr", j))
            bm = dram.tile([1, N], BF16, name=f"bm{j}")
            mrows[j] = mrow_pool.tile([1, N], BF16, tag="mrow",
                                      name=f"mrow{j}")
            nc.sync.dma_start(
                bm[:].rearrange("q (p jj) -> (q p) jj", jj=NJ), mr[:])
            nc.sync.dma_start(mrows[j][:], bm[:])

        def mlp_pair_a(g, j0, nj, jl, pair):
            """TENSOR/ACT half of one (cluster, 1024-pt) tile."""
            st = gstate[g]
            mrow = mrows[j0 + jl]
            ps2 = psum_pools["mlp"].tile([C, 2 * CHUNK], F32, tag="ps2")
            for half in range(2):
                ci = 2 * pair + half
                sl = slice(ci * CHUNK, (ci + 1) * CHUNK)
                qsl = slice(half * CHUNK, (half + 1) * CHUNK)
                h1 = h1_pool.tile([C, CHUNK], BF16, tag="h1")
                nc.scalar.activation(h1[:], P3[:, sl], ACTF.Relu,
                                     bias=st["U2b"][:, jl:jl + 1],
                                     scale=1.0)
                nc.tensor.matmul(ps2[:, qsl], W2r[:], h1[:],
                                 start=True, stop=False)
                nc.tensor.matmul(ps2[:, qsl], NB8[0:1, 0:128],
                                 mrow[:, sl], start=False,
                                 stop=True)
            rr = rr_pool.tile([C, 2 * CHUNK], BF16, tag="rr",
                              name=f"rr{g}_{jl}_{pair}")
            nc.scalar.copy(rr[:], ps2[:])
            st[("rr", jl, pair)] = rr

        def mlp_pair_b(g, nj, jl, pair):
            """Deferred DVE max-reduce of the pair's bf16 tile."""
            st = gstate[g]
            if pair == 0:
                st[("gp", jl)] = gp_pool.tile([C, 4], F32, tag="gparts",
                                              name=f"gp{g}_{jl}")
            rr = st.pop(("rr", jl, pair))
            nc.vector.tensor_reduce(st[("gp", jl)][:, pair:pair + 1], rr[:],
                                    axis=AX.X, op=ALU.max)

        def uf(g, j0, jl):
            st = gstate[g]
            nc.vector.tensor_reduce(
                G[:, j0 + jl:j0 + jl + 1], st[("gp", jl)][:],
                axis=AX.X, op=ALU.max)

        def boxes_group(j0, nj):
            """G cols -> box logits for clusters [j0, j0+nj)."""
            grel = bx_pool.tile([C, 8], F32R, tag="grel")
            nc.scalar.activation(grel[:, 0:nj], G[:, j0:j0 + nj],
                                 ACTF.Relu, bias=b2c[:], scale=1.0)
            ps_g3 = psum_pools["bx"].tile([C, 8], F32, tag="bx")
            nc.tensor.matmul(ps_g3[:, 0:nj], W3r[:], grel[:, 0:nj],
                             start=True, stop=True)
            g3 = bx_pool.tile([C, 8], F32R, tag="g3s")
            nc.scalar.activation(g3[:, 0:nj], ps_g3[:, 0:nj], ACTF.Relu,
                                 bias=b3c[:], scale=1.0)
            ps_g4 = psum_pools["bx"].tile([C, 8], F32, tag="bx")
            nc.tensor.matmul(ps_g4[:, 0:nj], W4r[:], g3[:, 0:nj],
                             start=True, stop=True)
            g4 = bx_pool.tile([C, 8], F32R, tag="g4s")
            nc.scalar.activation(g4[:, 0:nj], ps_g4[:, 0:nj], ACTF.Relu,
                                 bias=b4c[:], scale=1.0)
            ps_bx = psum_pools["bx"].tile([7, 8], F32, tag="bx")
            nc.tensor.matmul(ps_bx[:, 0:nj], Wfr[:], g4[:, 0:nj],
                             start=True, stop=False)
            nc.tensor.matmul(ps_bx[:, 0:nj], bfr[:], ones_1x16[:, 0:nj],
                             start=False, stop=True)
            nc.scalar.copy(BTmine[:, j0:j0 + nj], ps_bx[:, 0:nj])

        from collections import deque
        pending = deque()

        # startup work: P3 chunks
        for ci in range(NCHUNK):
            pending.append(lambda ci=ci: p3_chunk(ci))

        def enqueue_group(g, j0, j1, with_boxes=True):
            nj = j1 - j0
            pending.append(lambda: stage_a1(g, j0, nj))
            pending.append(lambda: stage_a2(g, j0, nj))
            for jl in range(nj):
                pending.append(lambda jl=jl: mask_relu(j0 + jl))
                pending.append(lambda jl=jl: mask_flat(j0 + jl))
            # interleave pair A (TE/ACT) and deferred B (DVE) items
            seq = []
            for jl in range(nj):
                for pair in range(4):
                    seq.append(("A", jl, pair))
            for idx, (_, jl, pair) in enumerate(seq):
                pending.append(
                    lambda jl=jl, pair=pair: mlp_pair_a(g, j0, nj, jl,
                                                        pair))
                # deferred reduce from ~2 items back
                if idx >= 2:
                    pjl, ppair = seq[idx - 2][1], seq[idx - 2][2]
                    pending.append(
                        lambda jl=pjl, pair=ppair: mlp_pair_b(g, nj, jl,
                                                              pair))
                    if ppair == 3:
                        pending.append(lambda jl=pjl: uf(g, j0, jl))
            for idx in (len(seq) - 2, len(seq) - 1):
                pjl, ppair = seq[idx][1], seq[idx][2]
                pending.append(
                    lambda jl=pjl, pair=ppair: mlp_pair_b(g, nj, jl, pair))
                if ppair == 3:
                    pending.append(lambda jl=pjl: uf(g, j0, jl))
            if with_boxes:
                pending.append(lambda: boxes_group(j0, nj))

        def pump(n):
            for _ in range(n):
                if pending:
                    pending.popleft()()

        # groups of 2 clusters: group g covers j in [2g, 2g+2), ready at
        # t = 16g+15.  The last two are singletons (j=14 @ t=119, j=15 @
        # t=127) with their boxes deferred to one even-width pass at the end.
        GROUP_AT = {}
        for g in range(7):
            GROUP_AT[16 * g + 15] = (g, 2 * g, 2 * g + 2, True)
        GROUP_AT[119] = (7, 14, 15, False)

        p3_closed = False
        for t in range(1, M):
            fps_iter(t)
            if t in GROUP_AT:
                g, j0, j1, wb = GROUP_AT[t]
                enqueue_group(g, j0, j1, with_boxes=wb)
            pump(2)
            if not p3_closed and t >= 8:
                p3_es.close()
                open_steady_psum()
                p3_closed = True
        if not p3_closed:
            p3_es.close()
            open_steady_psum()

        # ---- tail: AllGather 1 (j=0..13) hidden under remaining work ----
        bnc_in1 = dram.tile([7, 14], F32, name="bnc_in1")
        bnc_out1 = dram.tile([NCORES, 7 * 14], F32, name="bnc_out1")
        nc.sync.dma_start(bnc_in1[:], BTmine[:, 0:14])
        nc.gpsimd.collective_compute(
            "AllGather", mybir.AluOpType.bypass,
            replica_groups=[list(range(NCORES))],
            ins=[bnc_in1[:].opt()],
            outs=[bnc_out1[:].opt()],
        )

        # phantom distance update for center 127 -> cluster 15's d2 field
        sq128 = selq_of(127)
        nc.vector.tensor_tensor(
            pm2[:].rearrange("p (j c) -> p j c", c=3),
            pts96[:].rearrange("p (j c) -> p j c", c=3),
            sq128[:, 0:3].unsqueeze(1).broadcast_to([128, NJ, 3]),
            op=ALU.subtract)
        nc.vector.tensor_mul(sqt[:], pm2[:], pm2[:])
        nc.vector.tensor_reduce(
            ft1[:], sqt[:].rearrange("p (j c) -> p j c", c=3),
            axis=AX.X, op=ALU.add)
        nc.vector.scalar_tensor_tensor(
            ringd[15][:], ft1[:], wsel[:, 128:129], ringd[15][:],
            op0=ALU.mult, op1=ALU.add)

        # last cluster (j=15) + deferred boxes for j=14..16
        enqueue_group(8, 15, 16, with_boxes=False)
        pending.append(lambda: boxes_group(14, 2))
        while pending:
            pump(1)

        bnc_in2 = dram.tile([7, 2], F32, name="bnc_in2")
        bnc_out2 = dram.tile([NCORES, 7 * 2], F32, name="bnc_out2")
        nc.sync.dma_start(bnc_in2[:], BTmine[:, 14:16])
        nc.gpsimd.collective_compute(
            "AllGather", mybir.AluOpType.bypass,
            replica_groups=[list(range(NCORES))],
            ins=[bnc_in2[:].opt()],
            outs=[bnc_out2[:].opt()],
        )

        bx_es.close()
        mlp_es.close()
        sc_es.close()

        # reassemble: global cluster m = 8*j + k lives at
        # bounce_out[k, c*nj + (j - j0)]
        BTall = cp.tile([7, 128], F32)
        nc.sync.dma_start(
            BTall[:, 0:112].rearrange("c (j k) -> c j k", k=NCORES),
            bnc_out1[:].rearrange("k (c j) -> c j k", j=14),
        )
        nc.sync.dma_start(
            BTall[:, 112:128].rearrange("c (j k) -> c j k", k=NCORES),
            bnc_out2[:].rearrange("k (c j) -> c j k", j=2),
        )

        # ================= NMS =================
        nms_es = ExitStack()
        nms_psum = nms_es.enter_context(
            tc.tile_pool(name="nms_psum", bufs=1, space="PSUM"))
        # S7 = sigmoid(logits); BX = [sig | logits] transposed
        nc.scalar.activation(S14[0:7, :], BTall[:], ACTF.Sigmoid)
        ps_bxall = nms_psum.tile([128, 14], F32, tag="bxall")
        nc.tensor.transpose(ps_bxall[:, 0:7], S14[0:7, :], ident[0:7, 0:7])
        nc.tensor.transpose(ps_bxall[:, 7:14], BTall[:], ident[0:7, 0:7])
        nc.vector.tensor_copy(BX[:], ps_bxall[:])
        # cols of BX: 0 score-sig, 1..3 center, 4..6 dims, 7 score-logit
        nc.vector.scalar_tensor_tensor(lo3[:], BX[:, 4:7], -0.5, BX[:, 1:4],
                                       op0=ALU.mult, op1=ALU.add)
        nc.vector.scalar_tensor_tensor(hi3[:], BX[:, 4:7], 0.5, BX[:, 1:4],
                                       op0=ALU.mult, op1=ALU.add)
        nc.vector.tensor_mul(vol[:], BX[:, 4:5], BX[:, 5:6])
        nc.vector.tensor_mul(vol[:], vol[:], BX[:, 6:7])
        # PR = [lo3 | hi3 | vol | score-logit]
        nc.vector.tensor_copy(PR[:, 0:3], lo3[:])
        nc.vector.tensor_copy(PR[:, 3:6], hi3[:])
        nc.vector.tensor_copy(PR[:, 6:7], vol[:])
        nc.vector.tensor_copy(PR[:, 7:8], BX[:, 7:8])
        ps_tp = nms_psum.tile([8, 128], F32, tag="tp")
        nc.tensor.transpose(ps_tp[:], PR[:], ident[:])
        nc.vector.tensor_copy(TPs[:], ps_tp[:])
        # broadcast all 8 rows: psumB[:, r*128:(r+1)*128] = row r over parts
        psB = nms_psum.tile([128, 8 * 128], F32, tag="psB")
        for r in range(8):
            nc.tensor.matmul(psB[:, r * 128:(r + 1) * 128],
                             ER[:, r * 128:(r + 1) * 128],
                             TPs[:], start=True, stop=True)

        def colB(r):
            return psB[:, r * 128:(r + 1) * 128]

        wrk = nms_es.enter_context(tc.tile_pool(name="nms_wrk", bufs=1))
        inter = wrk.tile([128, 128], F32, tag="inter")
        tmpA = wrk.tile([128, 128], F32, tag="tmpA")
        tmpB = wrk.tile([128, 128], F32, tag="tmpB")
        for c in range(3):
            # min(hi_i, hi_j)
            nc.vector.tensor_scalar(tmpA[:], colB(3 + c), hi3[:, c:c + 1],
                                    None, op0=ALU.min)
            # max(lo_i, lo_j)
            nc.vector.tensor_scalar(tmpB[:], colB(c), lo3[:, c:c + 1], None,
                                    op0=ALU.max)
            # w = relu(minhi - maxlo)
            nc.vector.scalar_tensor_tensor(tmpA[:], tmpB[:], -1.0, tmpA[:],
                                           op0=ALU.mult, op1=ALU.add)
            nc.vector.tensor_scalar_max(tmpA[:], tmpA[:], 0.0)
            if c == 0:
                nc.vector.tensor_copy(inter[:], tmpA[:])
            else:
                nc.vector.tensor_mul(inter[:], inter[:], tmpA[:])
        # volsum = vol_i + vol_j + 1e-8
        nc.vector.tensor_scalar(tmpB[:], colB(6), vol[:], 1e-8, op0=ALU.add,
                                op1=ALU.add)
        # D = volsum - inter
        nc.vector.scalar_tensor_tensor(tmpB[:], inter[:], -1.0, tmpB[:],
                                       op0=ALU.mult, op1=ALU.add)
        # P_iou = (4*inter > D)
        nc.vector.scalar_tensor_tensor(tmpA[:], inter[:], 1.0 / NMS_THR,
                                       tmpB[:], op0=ALU.mult, op1=ALU.is_gt)
        # P_score[i,j] = score_j < score_i
        nc.vector.tensor_scalar(tmpB[:], colB(7), BX[:, 7:8], None,
                                op0=ALU.is_lt)
        nc.vector.tensor_mul(P_s[:], tmpA[:], tmpB[:])
        # Jacobi fixpoint: keep_j = !any_i P[i,j] keep_i  (bf16 matmuls)
        nc.vector.memset(keep[:], 1.0)
        ps_k = nms_psum.tile([128, 1], F32, tag="kps")
        for it in range(NMS_ITERS):
            nc.tensor.matmul(ps_k[:], P_s[:], keep[:], start=True, stop=True)
            nc.vector.tensor_scalar(keep[:], ps_k[:], 0.5, None,
                                    op0=ALU.is_lt)
        nc.vector.tensor_copy(keepf[:], keep[:])
        # out = coords * keep
        nc.vector.tensor_scalar(outt[:], BX[:, 1:7], keepf[:], None,
                                op0=ALU.mult)
        nc.sync.dma_start(d_out.ap(), outt[:])

        nms_es.close()
        es.close()

    nc.compile()
    return nc


def _prep_inputs(vote_points, vote_features, W1, b1, W2, b2, W3, b3, W4, b4,
                 Wf, bf):
    """Pure layout transforms of the full inputs -> per-core input maps."""
    f32 = np.float32
    pts = np.ascontiguousarray(vote_points, dtype=f32)
    feat = np.ascontiguousarray(vote_features, dtype=f32)
    base = {
        "pts96": pts.reshape(128, 96).copy(),
        "pT": pts.T.copy(),
        "featT": feat.T.copy(),
        "W1a": np.ascontiguousarray(W1[:3], f32),
        "W1am2": np.ascontiguousarray(W1[:3] * -2.0, f32),
        "W1b": np.ascontiguousarray(W1[3:], f32),
        "W2": np.ascontiguousarray(W2, f32),
        "W3": np.ascontiguousarray(W3, f32),
        "W4": np.ascontiguousarray(W4, f32),
        "Wf": np.ascontiguousarray(Wf, f32),
        "b1r": np.ascontiguousarray(b1, f32).reshape(1, C),
        "b2c": np.ascontiguousarray(b2, f32).reshape(C, 1),
        "b3c": np.ascontiguousarray(b3, f32).reshape(C, 1),
        "b4c": np.ascontiguousarray(b4, f32).reshape(C, 1),
        "bfr": np.ascontiguousarray(bf, f32).reshape(1, 7),
    }
    in_maps = []
    for k in range(NCORES):
        m = dict(base)
        sel = np.zeros((128, MC), f32)
        for j in range(MC):
            sel[NCORES * j + k, j] = 1.0
        m["sel16"] = sel
        w = np.zeros((1, 129), f32)
        for t in range(1, 129):
            if (t - 1) % NCORES == k:
                w[0, t] = 1.0
        m["wsel"] = np.repeat(w, 128, axis=0)
        in_maps.append(m)
    return in_maps


def kernel(**inputs):
    from concourse.bass_utils import run_bass_kernel_spmd

    if "nc" not in _cache:
        _cache["nc"] = _build(debug=False)
    nc = _cache["nc"]
    in_maps = _prep_inputs(**inputs)
    res = run_bass_kernel_spmd(nc, in_maps, core_ids=list(range(NCORES)))
    out = np.asarray(res.results[0]["out"], dtype=np.float32)
    return out
